# revision 1
# baseline (speedup 1.0000x reference)
"""Contrastive (NT-Xent) loss kernel for 8 Trainium2 NeuronCores.

Math (reference): z = l2norm(concat(proj_1, proj_2)) [8192,128];
sim = z @ z.T; loss = mean_i( log(sum_{j!=i} exp(2*sim_ij)) - 2*pos_i ).

Sharding: rows of the 8192x8192 sim matrix are split 1024/core. Each core
receives the full rep matrix rotated by core*1024 rows (host-side layout
only), so its own rows are always local columns [0,1024) and the positive
partners are at [4096,5120) -- one identical SPMD program, static offsets.
Each core emits one partial scalar; the host sums 8 floats.

Device pipeline (per core), interleaved in groups of 2048 columns so the
ACT engine (the bottleneck: exp at 1 elem/cycle/lane) starts ~5us in:
  group g: DMA 4 natural bf16 chunks -> row norms (DVE, fp32 accum) ->
  1/norm via ln/exp (compact [128,16], ACT) -> per-row scale (DVE) ->
  PE-transpose into normalized bf16 X^T columns -> main quarter g:
  8 m-tiles x (4 bf16 matmuls -> fused exp+row-sum ACT op, [128,2048]
  PSUM). bf16 input halves the DMA head; loss rel err stays ~1e-6.
Then log-denominator, positives dot, partition-sum matmuls, one scalar out.
"""

import ml_dtypes
import numpy as np

import concourse.bass as bass
import concourse.tile as tile
from concourse import bacc, mybir
from concourse.bass_utils import run_bass_kernel_spmd
from concourse.hw_specs import get_activation_tables
from concourse.masks import make_identity

B = 4096
D = 128
N2 = 2 * B            # 8192 total rows
NCORES = 8
RPC = N2 // NCORES    # 1024 rows per core
MT = RPC // 128       # 8 m-tiles of 128 rows
NCH = N2 // 512       # 16 column chunks of 512
NG = 4                # groups of 4 chunks (2048 cols)
TEMP = 0.5
E2 = float(np.exp(1.0 / TEMP))   # exp(sim_ii / T) with sim_ii == 1

F32 = mybir.dt.float32
F32R = mybir.dt.float32r
BF16 = mybir.dt.bfloat16
AX = mybir.AxisListType
OP = mybir.AluOpType
AF = mybir.ActivationFunctionType

LAST_RESULT = None  # BassKernelResults of the most recent run (for test.py)


def _build_nc():
    nc = bacc.Bacc("TRN2", target_bir_lowering=False)
    xn_d = nc.declare_dram_parameter("xn", [N2, D], BF16, isOutput=False)
    out_d = nc.declare_dram_parameter("out", [1, 1], F32, isOutput=True)

    # Pre-place the one ACT table set that covers both Ln and Exp, so the
    # greedy per-func chooser never inserts mid-kernel table switches.
    table_names = list(get_activation_tables(nc.m.arch).keys())
    combined_id = table_names.index("natural_log_exp_and_others")

    with tile.TileContext(nc) as tc:
        with (
            tc.tile_pool(name="big", bufs=1) as big,
            tc.tile_pool(name="work", bufs=3) as work,
            tc.tile_pool(name="scr", bufs=2) as scr,
            tc.tile_pool(name="ps", bufs=2, space="PSUM") as ps,
        ):
            nc.scalar.add_instruction(mybir.InstLoadActFuncSet(
                name=nc.get_next_instruction_name(), ins=[], outs=[],
                act_func_set_id=combined_id))

            xn_all = big.tile([128, 64, 128], BF16, tag="xn")  # [p, j, d]: row j*128+p
            xhat = big.tile([D, N2], BF16, tag="xhat")         # normalized reps^T
            ns_c = big.tile([128, 64], F32, tag="ns")         # |row|^2 compact
            lnn = big.tile([128, 64], F32, tag="lnn")
            s_c = big.tile([128, 64], F32, tag="s")           # 1/|row| compact
            ones_col = big.tile([128, 1], F32, tag="ones_col")
            rs_all = big.tile([128, MT * NG], F32, tag="rs")  # exp row-sums (m, g)
            ident = big.tile([128, 128], BF16, tag="ident")
            pacc = big.tile([128, 1], F32, tag="pacc")

            def prep_group(g):
                """DMA 4 natural chunks, row norms, 1/norm, scale, transpose
                into xhat columns [2048g, 2048(g+1))."""
                for c in range(4 * g, 4 * g + 4):
                    # alternate the two HWDGE queues (SP / ACT)
                    eng = nc.sync if c % 2 == 0 else nc.scalar
                    eng.dma_start(
                        out=xn_all[:, c * 4:(c + 1) * 4, :],
                        in_=xn_d[c * 512:(c + 1) * 512, :].rearrange(
                            "(t p) d -> p t d", p=128
                        ),
                    )
                    # normsq per 128-row block: fused square + row-sum
                    for j in range(4):
                        jj = c * 4 + j
                        sqs = work.tile([128, 128], F32, tag="sqs")
                        blk = xn_all[:, jj, :]
                        nc.vector.scalar_tensor_tensor(
                            out=sqs, in0=blk, scalar=1.0, in1=blk,
                            op0=OP.mult, op1=OP.mult,
                            accum_out=ns_c[:, jj:jj + 1],
                        )
                # 1/norm = exp(-0.5*ln(normsq)); same ACT table set as exp.
                # group 0 is the latency-critical head: do it per chunk-pair
                # so the chain doesn't wait for all 4 chunk DMAs.
                subs = 2 if g == 0 else 1
                # high priority: these tiny ops must not queue behind the
                # previous quarter's exp stream on ACT (they gate this
                # group's scale->transpose chain and its PSUM slot release)
                with tc.high_priority():
                    for i in range(subs):
                        w = 16 // subs
                        gsl = slice(16 * g + i * w, 16 * g + (i + 1) * w)
                        nc.scalar.activation(
                            out=lnn[:, gsl], in_=ns_c[:, gsl], func=AF.Ln
                        )
                        nc.scalar.activation(
                            out=s_c[:, gsl], in_=lnn[:, gsl], func=AF.Exp,
                            scale=-0.5,
                        )
                # scale rows, PE-transpose into xhat columns (bf16)
                tp = ps.tile([128, 2048], BF16, tag="ps")
                for c in range(4 * g, 4 * g + 4):
                    xsc = work.tile([128, 4, 128], BF16, tag="xsc")
                    nc.vector.tensor_mul(
                        xsc,
                        xn_all[:, c * 4:(c + 1) * 4, :],
                        s_c[:, c * 4:(c + 1) * 4].broadcast_to([128, 4, 128]),
                    )
                    for j in range(4):
                        nc.tensor.transpose(
                            tp[:, (c % 4) * 512 + j * 128:(c % 4) * 512 + (j + 1) * 128],
                            xsc[:, j, :],
                            ident[:],
                        )
                    nc.vector.tensor_copy(
                        xhat[:, c * 512:(c + 1) * 512],
                        tp[:, (c % 4) * 512:(c % 4 + 1) * 512],
                    )
                if g == 2:
                    # positives dot (needs xhat chunks 0,1 and 8,9)
                    prod = scr.tile([128, RPC], F32, tag="scr")
                    nc.vector.scalar_tensor_tensor(
                        out=prod,
                        in0=xhat[:, 0:RPC],
                        scalar=1.0,
                        in1=xhat[:, B:B + RPC],
                        op0=OP.mult,
                        op1=OP.mult,
                        accum_out=pacc,
                    )

            def quarter_half(g, half):
                """4 m-tiles of main work on columns [2048g, 2048(g+1))."""
                for m in range(4 * half, 4 * half + 4):
                    pst = ps.tile([128, 2048], F32, tag="ps")
                    lhsT = xhat[:, m * 128:(m + 1) * 128]
                    for s4 in range(4):
                        col = g * 2048 + s4 * 512
                        nc.tensor.matmul(
                            pst[:, s4 * 512:(s4 + 1) * 512],
                            lhsT=lhsT,
                            rhs=xhat[:, col:col + 512],
                            start=True,
                            stop=True,
                        )
                    sc = scr.tile([128, 2048], BF16, tag="scr")
                    nc.scalar.activation(
                        out=sc,
                        in_=pst,
                        func=AF.Exp,
                        scale=1.0 / TEMP,
                        accum_out=rs_all[:, m * NG + g:m * NG + g + 1],
                    )

            nc.vector.memset(ones_col, 1.0)
            make_identity(nc, ident[:])

            # interleave: group g+1 prep emitted mid-quarter-g so its DMAs,
            # DVE work and PSUM slot use hide under the ACT exp stream
            prep_group(0)
            quarter_half(0, 0)
            prep_group(1)
            quarter_half(0, 1)
            quarter_half(1, 0)
            prep_group(2)
            quarter_half(1, 1)
            quarter_half(2, 0)
            prep_group(3)
            quarter_half(2, 1)
            quarter_half(3, 0)
            quarter_half(3, 1)

            # ---- finals ----
            rowsum = big.tile([128, MT], F32, tag="rowsum")
            nc.vector.tensor_reduce(
                out=rowsum,
                in_=rs_all[:].rearrange("p (m g) -> p m g", g=NG),
                axis=AX.X,
                op=OP.add,
            )
            den = big.tile([128, MT], F32, tag="den")
            nc.vector.tensor_scalar_add(out=den, in0=rowsum, scalar1=-E2)
            logden = big.tile([128, MT], F32, tag="logden")
            nc.scalar.activation(out=logden, in_=den, func=AF.Ln)
            ldps = ps.tile([1, MT], F32, tag="ps")
            nc.tensor.matmul(ldps, lhsT=ones_col, rhs=logden, start=True, stop=True)
            pps = ps.tile([1, 1], F32, tag="ps")
            nc.tensor.matmul(pps, lhsT=ones_col, rhs=pacc, start=True, stop=True)

            l1 = big.tile([1, 1], F32, tag="l1")
            nc.vector.tensor_reduce(out=l1, in_=ldps, axis=AX.X, op=OP.add)
            t2 = big.tile([1, 1], F32, tag="t2")
            nc.vector.tensor_scalar_mul(out=t2, in0=pps, scalar1=-2.0)
            res = big.tile([1, 1], F32, tag="res")
            nc.vector.tensor_add(res, l1, t2)
            nc.vector.tensor_scalar_mul(out=res, in0=res, scalar1=1.0 / N2)
            nc.sync.dma_start(out=out_d[:, :], in_=res)

    nc.compile()
    return nc


_NC = None


def kernel(proj_1: np.ndarray, proj_2: np.ndarray) -> np.ndarray:
    global _NC, LAST_RESULT
    import os

    reps = np.concatenate(
        [np.asarray(proj_1, np.float32), np.asarray(proj_2, np.float32)], axis=0
    )
    assert reps.shape == (N2, D)

    in_maps = [
        {"xn": np.ascontiguousarray(np.roll(reps, -c * RPC, axis=0)).astype(ml_dtypes.bfloat16)}
        for c in range(NCORES)
    ]

    if _NC is None:
        _NC = _build_nc()

    trace = bool(os.environ.get("CONTRASTIVE_TRACE"))
    result = run_bass_kernel_spmd(
        _NC, in_maps, core_ids=list(range(NCORES)), trace=trace
    )
    LAST_RESULT = result
    total = sum(float(r["out"][0, 0]) for r in result.results)
    return np.float32(total)



# revision 7
# speedup vs baseline: 1.1521x; 1.1521x over previous
"""Contrastive (NT-Xent) loss kernel for 8 Trainium2 NeuronCores.

Math (reference): z = l2norm(concat(proj_1, proj_2)) [8192,128];
sim = z @ z.T; loss = mean_i( log(sum_{j!=i} exp(2*sim_ij)) - 2*pos_i ).

Sharding: rows of the 8192x8192 sim matrix are split 1024/core. Each core
receives the full rep matrix rotated by core*1024 rows (host-side layout
only), so its own rows are always local rows [0,1024) and the positive
partners are at columns [4096,5120) -- one identical SPMD program, static
offsets. Each core emits one partial scalar; the host sums 8 floats.

Device pipeline (per core):
  prep (per 512-row chunk): DMA fp16 natural layout -> row normsq
  (Pool square + DVE reduce) -> 1/norm = exp(-.5*ln(ns)) (ACT, batched)
  -> diag(1/n) built by one Pool affine_select -> PE transpose-matmul
  in_^T @ diag fuses the row scaling into the transpose -> Pool copy
  PSUM->SBUF xhat[d, rows].
  main: for each 128-row m-tile x column segment: PE matmuls (512-col
  pieces) then either ACT exp (exact, fused row-sum accumulator) or a
  custom 7-stage DVE op computing ((a*s+b)*s+c)^8 ~ exp(2s) with fused
  row-sum -- the two engines drain separate PSUM banks in parallel
  (ACT: 2x3-bank slots, DVE: 2x1-bank slots). The quad^8 fit keeps the
  log-denominator error ~2e-5. Positives are computed exactly from xhat.
Then log-denominator, positives dot, partition-sum matmuls, one scalar.
"""

import os
from operator import add as _op_add

import numpy as np

import concourse.bass as bass
import concourse.tile as tile
from concourse import bacc, mybir
from concourse.bass_utils import run_bass_kernel_spmd
from concourse.hw_specs import get_activation_tables

import concourse.dve_ops as dve_ops
from concourse.dve_ops import DveOp
from concourse.dve_spec import Spec, Src0, C0, C1, C2, Zero, sq as _sq

B = 4096
D = 128
N2 = 2 * B            # 8192 total rows
NCORES = 8
RPC = N2 // NCORES    # 1024 rows per core
NCH = 16              # 512-row chunks of the rep matrix
TEMP = 0.5
E2 = float(np.exp(1.0 / TEMP))   # exp(sim_ii / T) with sim_ii == 1
NSEG = 12             # rs_all slots per m-tile (padded; memset to 0)

# exp(2s) ~ ((QA*s + QB)*s + QC)^8, weighted fit of e^(s/4) for the
# N(0, 1/128) similarity distribution (log-denominator error ~2e-5).
QA = 0.030779760361204535
QB = 0.25013234394490524
QC = 1.0000028534022587

F32 = mybir.dt.float32
F16 = mybir.dt.float16
AX = mybir.AxisListType
OP = mybir.AluOpType
AF = mybir.ActivationFunctionType

LAST_RESULT = None  # BassKernelResults of the most recent run (for test.py)

USE_DVE_EXP = not bool(os.environ.get("CONTRASTIVE_NO_DVE"))

EXP_OP_NAME = "EXP2S_QUAD8_REDUCE_ANT"
_EXP_OP = None


def _register_exp_op():
    """Register the custom DVE op (idempotent)."""
    global _EXP_OP
    if _EXP_OP is not None:
        return _EXP_OP
    for o in dve_ops.OPS:
        if o.name == EXP_OP_NAME:
            _EXP_OP = o
            return o

    def _ref(in0, in1, c0, c1, c2):
        x = in0.astype(np.float32)
        q = ((np.float32(c0) * x + np.float32(c1)) * x + np.float32(c2)).astype(
            np.float32
        )
        for _ in range(3):
            q = (q * q).astype(np.float32)
        acc = (
            q.reshape(q.shape[0], -1)
            .astype(np.float32)
            .sum(axis=-1, keepdims=True, dtype=np.float32)
        )
        return q, acc

    body = _sq(_sq(_sq((Src0 * C0 + C1) * Src0 + C2)))
    spec = Spec(body=body, accum=_op_add, accum_init=Zero, reference=_ref)
    op = DveOp(
        EXP_OP_NAME,
        spec,
        subdim=False,
        uops_sha={"v3": "6b486dcc8d231292", "v4": "4e58416635560d84"},
    )
    dve_ops.OPS.append(op)
    dve_ops.CUSTOM_DVE_SPECS[EXP_OP_NAME] = spec
    dve_ops._SUB_OPCODE_FOR_NAME[EXP_OP_NAME] = (
        dve_ops._CUSTOM_DVE_ROW_BASE + len(dve_ops.OPS) - 1
    )
    _EXP_OP = op
    return op


def _patterns(r, m):
    """Column-segment pattern for region r (2048 cols), m-tile m.

    ('A', w): exact exp on ACT; ('D', w): quad^8 approx on DVE."""
    if not USE_DVE_EXP:
        return [("A", 1536), ("A", 512)]
    if r == 0:
        return [("D", 512), ("A", 1536)]
    if r in (1, 2):
        return [("D", 512), ("D", 512), ("A", 1024)]
    if r == 3 and m < 4:
        return [("D", 512), ("D", 512), ("A", 1024)]
    return [("D", 512), ("A", 1536)]


def _build_nc():
    exp_op = _register_exp_op()
    nc = bacc.Bacc("TRN2", target_bir_lowering=False)
    xn_d = nc.declare_dram_parameter("xn", [N2, D], F16, isOutput=False)
    out_d = nc.declare_dram_parameter("out", [1, 1], F32, isOutput=True)

    table_names = list(get_activation_tables(nc.m.arch).keys())
    combined_id = table_names.index("natural_log_exp_and_others")

    with tile.TileContext(nc) as tc:
        with (
            tc.tile_pool(name="big", bufs=1) as big,
            tc.tile_pool(name="sqp", bufs=3) as sqp,
            tc.tile_pool(name="dgp", bufs=3) as dgp,
            tc.tile_pool(name="aop", bufs=2) as aop,
            tc.tile_pool(name="dop", bufs=2) as dop,
            tc.tile_pool(name="pact", bufs=2, space="PSUM") as pact,
            tc.tile_pool(name="pdve", bufs=2, space="PSUM") as pdve,
        ):
            nc.scalar.add_instruction(mybir.InstLoadActFuncSet(
                name=nc.get_next_instruction_name(), ins=[], outs=[],
                act_func_set_id=combined_id))

            xn_all = big.tile([128, 4 * NCH, 128], F16, tag="xn")  # [p, blk, d]
            xhat = big.tile([D, N2], F16, tag="xhat")              # scaled reps^T
            ns_c = big.tile([128, 4 * NCH], F32, tag="ns")         # |row|^2
            lnn = big.tile([128, 4 * NCH], F32, tag="lnn")
            s_c = big.tile([128, 4 * NCH], F32, tag="s")           # 1/|row|
            rs_all = big.tile([128, 8 * NSEG], F32, tag="rs")      # row-sum parts
            ones32 = big.tile([128, 1], F32, tag="ones32")
            pacc = big.tile([128, 1], F32, tag="pacc")
            prod = big.tile([128, RPC], F32, tag="prod")
            rowsum = big.tile([128, 8], F32, tag="rowsum")
            den = big.tile([128, 8], F32, tag="den")
            logden = big.tile([128, 8], F32, tag="logden")
            l1 = big.tile([1, 1], F32, tag="l1")
            t2 = big.tile([1, 1], F32, tag="t2")
            res = big.tile([1, 1], F32, tag="res")

            nc.gpsimd.memset(rs_all, 0.0)
            nc.vector.memset(ones32, 1.0)

            # all 16 chunk loads on the SP queue, natural [p, blk, d] layout
            for c in range(NCH):
                nc.sync.dma_start(
                    out=xn_all[:, c * 4:(c + 1) * 4, :],
                    in_=xn_d[c * 512:(c + 1) * 512, :].rearrange(
                        "(t p) d -> p t d", p=128
                    ),
                )

            def prep_norm(c):
                """squares (Pool) + per-block row-reduce (DVE) for chunk c."""
                sqt = sqp.tile([128, 4, 128], F16, tag="sq")
                nc.gpsimd.tensor_mul(
                    sqt, xn_all[:, c * 4:(c + 1) * 4, :],
                    xn_all[:, c * 4:(c + 1) * 4, :],
                )
                nc.vector.tensor_reduce(
                    out=ns_c[:, c * 4:(c + 1) * 4], in_=sqt[:],
                    axis=AX.X, op=OP.add,
                )

            def lnexp(clo, chi):
                """1/norm = exp(-0.5*ln(normsq)) for chunks [clo, chi)."""
                sl = slice(clo * 4, chi * 4)
                nc.scalar.activation(out=lnn[:, sl], in_=ns_c[:, sl], func=AF.Ln)
                nc.scalar.activation(
                    out=s_c[:, sl], in_=lnn[:, sl], func=AF.Exp, scale=-0.5,
                )

            def make_diag(c):
                """diag(1/n) for chunk c's 4 blocks in one Pool affine_select."""
                dg = dgp.tile([128, 4, 128], F16, tag="dg")
                nc.gpsimd.affine_select(
                    out=dg,
                    in_=s_c[:, c * 4:(c + 1) * 4].broadcast_to([128, 4, 128]),
                    compare_op=OP.is_equal,
                    fill=0.0,
                    base=0,
                    pattern=[[0, 4], [-1, 128]],
                    channel_multiplier=1,
                )
                return dg

            def tp_chunks(chunks, pool):
                """Scaled transpose of chunks into xhat via one borrowed
                PSUM slot: out = xn_blk^T @ diag(1/n)."""
                dgs = [make_diag(c) for c in chunks]
                w = 512 * len(chunks)
                tpt = pool.tile([128, w], F32, tag="pa" if pool is pact else "pd")
                for i, c in enumerate(chunks):
                    for j in range(4):
                        # scaled transpose as a plain matmul: with
                        # lhsT = x_blk [rows, d] and rhs = diag(1/n),
                        # out[d, r] = x[r, d] / n_r  (transpose mode would
                        # reject a non-permutation rhs).
                        nc.tensor.matmul(
                            tpt[:, i * 512 + j * 128:i * 512 + (j + 1) * 128],
                            lhsT=xn_all[:, c * 4 + j, :],
                            rhs=dgs[i][:, j, :],
                            start=True,
                            stop=True,
                        )
                for i, c in enumerate(chunks):
                    # only DVE can move PSUM->SBUF (GPSIMD and DMA cannot
                    # read PSUM; ACT is the bottleneck engine)
                    nc.vector.tensor_copy(
                        xhat[:, c * 512:(c + 1) * 512],
                        tpt[:, i * 512:(i + 1) * 512],
                    )

            # head: chunks 0..3 prepped before the main stream
            for c in (0, 1):
                prep_norm(c)
            lnexp(0, 2)
            for c in (2, 3):
                prep_norm(c)
            tp_chunks((0, 1), pact)
            lnexp(2, 4)
            tp_chunks((2, 3), pact)

            # chunk-pair preps interleaved into the region-0 m loop:
            # (emitted after the given m's segments)
            prep_after = {
                1: ((4, 5), pdve, 1),   # pair, tp pool, chunks per tp use
                2: ((6, 7), pdve, 1),
                3: ((8, 9), pdve, 1),
                4: ((10, 11), pact, 2),
                5: ((12, 13), pact, 2),
                6: ((14, 15), pact, 2),
            }

            def emit_prep(pair, pool, per_use):
                a, b = pair
                prep_norm(a)
                prep_norm(b)
                lnexp(a, b + 1)
                if per_use == 1:
                    tp_chunks((a,), pool)
                    tp_chunks((b,), pool)
                else:
                    tp_chunks((a, b), pool)

            def segment(m, col, kind, w, rs_idx):
                lhsT = xhat[:, m * 128:(m + 1) * 128]
                pool = pact if kind == "A" else pdve
                pst = pool.tile([128, w], F32, tag="pa" if kind == "A" else "pd")
                for o in range(0, w, 512):
                    pw = min(512, w - o)
                    nc.tensor.matmul(
                        pst[:, o:o + pw],
                        lhsT=lhsT,
                        rhs=xhat[:, col + o:col + o + pw],
                        start=True,
                        stop=True,
                    )
                acc = rs_all[:, m * NSEG + rs_idx:m * NSEG + rs_idx + 1]
                if kind == "A":
                    ot = aop.tile([128, w], F16, tag="ao")
                    nc.scalar.activation(
                        out=ot, in_=pst, func=AF.Exp, scale=1.0 / TEMP,
                        accum_out=acc,
                    )
                else:
                    ot = dop.tile([128, w], F16, tag="do")
                    nc.vector._custom_dve(
                        exp_op, out=ot, in0=pst[:],
                        s0=QA, s1=QB, imm2=QC, accum_out=acc,
                    )

            def finals_reduce(mlo, mhi):
                sl = slice(mlo, mhi)
                nc.vector.tensor_reduce(
                    out=rowsum[:, sl],
                    in_=rs_all[:].rearrange("p (m s) -> p m s", s=NSEG)[:, sl, :],
                    axis=AX.X,
                    op=OP.add,
                )
                nc.vector.tensor_scalar_add(
                    out=den[:, sl], in0=rowsum[:, sl], scalar1=-E2
                )
                nc.scalar.activation(
                    out=logden[:, sl], in_=den[:, sl], func=AF.Ln
                )

            for r in range(4):
                for m in range(8):
                    rs_idx = 0
                    col = r * 2048
                    for kind, w in _patterns(r, m):
                        segment(m, col, kind, w, r * 3 + rs_idx)
                        col += w
                        rs_idx += 1
                    if r == 0 and m in prep_after:
                        emit_prep(*prep_after[m])
                    if r == 0 and m == 4:
                        # positives dot: local rows x cols [B, B+RPC)
                        nc.vector.scalar_tensor_tensor(
                            out=prod,
                            in0=xhat[:, 0:RPC],
                            scalar=1.0,
                            in1=xhat[:, B:B + RPC],
                            op0=OP.mult,
                            op1=OP.mult,
                            accum_out=pacc,
                        )
            # finals: log-denominators, partition sums, one scalar out
            finals_reduce(0, 8)
            ldps = pdve.tile([1, 16], F32, tag="pd")
            nc.tensor.matmul(
                ldps[0:1, 0:8], lhsT=ones32, rhs=logden, start=True, stop=True
            )
            nc.tensor.matmul(
                ldps[0:1, 8:9], lhsT=ones32, rhs=pacc, start=True, stop=True
            )
            nc.vector.tensor_reduce(
                out=l1, in_=ldps[0:1, 0:8], axis=AX.X, op=OP.add
            )
            nc.vector.tensor_scalar_mul(out=t2, in0=ldps[0:1, 8:9], scalar1=-2.0)
            nc.vector.tensor_add(res, l1, t2)
            nc.vector.tensor_scalar_mul(out=res, in0=res, scalar1=1.0 / N2)
            nc.sync.dma_start(out=out_d[:, :], in_=res)

    nc.compile()
    return nc


_NC = None


def kernel(proj_1: np.ndarray, proj_2: np.ndarray) -> np.ndarray:
    global _NC, LAST_RESULT

    reps = np.concatenate(
        [np.asarray(proj_1, np.float32), np.asarray(proj_2, np.float32)], axis=0
    )
    assert reps.shape == (N2, D)

    in_maps = [
        {"xn": np.ascontiguousarray(np.roll(reps, -c * RPC, axis=0)).astype(np.float16)}
        for c in range(NCORES)
    ]

    if _NC is None:
        _NC = _build_nc()

    trace = bool(os.environ.get("CONTRASTIVE_TRACE"))
    result = run_bass_kernel_spmd(
        _NC, in_maps, core_ids=list(range(NCORES)), trace=trace
    )
    LAST_RESULT = result
    total = sum(float(r["out"][0, 0]) for r in result.results)
    return np.float32(total)
